# revision 3
# baseline (speedup 1.0000x reference)
"""Multi-scale patch pooling (gather + segment-mean) for CLIP-AD on 8 trn2 cores.

Reference, per batch element b:
    large[b, g, :] = mean over l of tokens[b, large_mask[l, g], :]   (9-elt mean, 169 groups)
    mid[b, g, :]   = mean over l of tokens[b, mid_mask[l, g], :]     (4-elt mean, 196 groups)
    cls[b, 0, :]   = mean over t of tokens[b, t, :]                  (225-elt mean)
    out = concat([large, mid, cls], axis=1)                          # [B, 366, D]

Per batch out_b = W @ tokens_b with W a [366, 225] membership matrix built
host-side from the masks (handles arbitrary/duplicate indices; 1/L folded in).
Everything bf16 on device (~2.6e-3 rel err vs 2e-2 budget; halves HBM bytes).

DMA engine-assignment rules (measured on this silicon, see session notes):
  * HBM->SBUF: a transfer with EXACTLY 128 partitions spreads its descriptors
    over all 16 SDMA engines (8 descs each, partition->port swizzle). Any
    sub-128-partition transfer is pinned to ONE engine (~27 GB/s).
  * SBUF->HBM: engine = DRAM address granule (~1.31 MB), partition count and
    striding irrelevant.

Kernel layout exploits both rules:
  * K = 225 tokens split as two 128-row chunks: A = tokens 0..127, B = tokens
    97..224 (31-row overlap; W rows for tokens 97..127 are ZEROED in chunk B).
    Every token load is one [128, 14336 B] DMA -> all 16 engines. Costs +13.8%
    input bytes, buys ~16x load bandwidth.
  * Per quad (4 batches) one load: row p = [A: 4 batches x 896 | B: ...].
  * Stores are split into 4 partition-quarters (31/31/30/30 rows), each
    written to its own 1.31 MB-aligned DRAM region (out[4q+j]) so consecutive
    quarter-stores land on consecutive SDMA engines, cycling all 16.
  * Loads issue on gpsimd (SWDGE), stores on nc.sync (HWDGE qSP) - separate
    in-order queues, so a store waiting on PSUM evacuation never blocks loads.
  * Matmul: per (half, mi) accumulate 2 k-chunks into [122, 896] f32 PSUM
    (bank-aligned 512+384 N tiles), evacuate alternating DVE/ACT with
    f32->bf16 cast into a [122, 4*3*896] o tile, 4-deep o pool.

Sharding: pure data parallel on batch - 64 batches (16 quads) per core.
"""

import numpy as np

B, T, D = 512, 225, 896
GL, LL = 169, 9
GM, LM = 196, 4
G = GL + GM + 1  # 366
N_CORES = 8
BP = B // N_CORES  # 64
QB = 4             # batches per quad
NQ = BP // QB      # 16 quads per core

KC = 128                      # k-chunk partition count (both chunks)
KB0 = T - KC                  # 97: chunk B = tokens 97..224
MP = G // 3                   # 122 partitions per m-tile (groups strided by 3)
_N_TILES = ((0, 512), (512, 384))
ROWE = 2 * QB * D             # packed row elems per partition (7168)
OCOL = QB * 3 * D             # o-tile cols per quad (10752 elems)
QROWS = 61                    # out region rows: 61*21504B = 1.31MB DRAM granule
_Q_SPLITS = ((0, 31), (31, 31), (62, 30), (92, 30))  # partition quarters of 122

NTOK = 6   # token quad slots
NOB = 4    # o-tile slots
LOOK = 4   # quads of load lookahead

_CACHE = {}


def _get_nc():
    if "nc" in _CACHE:
        return _CACHE["nc"]
    from contextlib import ExitStack

    import concourse.bacc as bacc
    import concourse.mybir as mybir
    import concourse.tile as tile

    f32 = mybir.dt.float32
    bf16 = mybir.dt.bfloat16

    nc = bacc.Bacc("TRN2", target_bir_lowering=False, debug=False)
    # tokq[q, p, ki*4D + b*D + d] = bf16 token (4q+b, ki ? 97+p : p, d)
    tokq = nc.dram_tensor("tokq", [NQ, KC, ROWE], bf16, kind="ExternalInput").ap()
    # w01T[ki, p, mi*122 + m] = weight of group perm(3m+mi) at token (ki?97+p:p)
    # (rows 0..30 of ki=1 are zero: tokens 97..127 counted in chunk A)
    w01T = nc.dram_tensor("w01T", [2, KC, G], bf16, kind="ExternalInput").ap()
    # out[4q+j] = one 1.31MB DRAM granule per partition-quarter store
    out = nc.dram_tensor("out", [NQ * 4, QROWS, OCOL], bf16, kind="ExternalOutput").ap()

    with tile.TileContext(nc) as tc:
        with ExitStack() as ctx:
            tok_pools = [
                ctx.enter_context(tc.tile_pool(name=f"tokp{s}", bufs=1))
                for s in range(NTOK)
            ]
            obp = ctx.enter_context(tc.tile_pool(name="ob", bufs=NOB))
            wp = ctx.enter_context(tc.tile_pool(name="w", bufs=1))
            psp = ctx.enter_context(tc.tile_pool(name="ps", bufs=4, space="PSUM"))

            # Warm-up ops: first ACT/DVE instructions pick up table-load waits
            # in lowering; give them dummies with no cross-engine deps.
            warm = wp.tile([128, 2], f32, tag="warm")
            nc.gpsimd.memset(warm[:], 0.0)
            nc.scalar.activation(
                warm[:], warm[:], mybir.ActivationFunctionType.Copy
            )
            nc.vector.tensor_copy(warm[:], warm[:])

            w_sb = []
            for ki in range(2):
                wt = wp.tile([128, G], bf16, tag=f"w{ki}")
                nc.gpsimd.dma_start(wt[:, :], w01T[ki, :, :])
                w_sb.append(wt)

            tks = {}

            def emit_load(q):
                tk = tok_pools[q % NTOK].tile([128, ROWE], bf16, name="tok", tag="tok")
                nc.gpsimd.dma_start(tk[:, :], tokq[q, :, :])
                tks[q] = tk

            for q in range(min(LOOK, NQ)):
                emit_load(q)
            loaded = min(LOOK, NQ)

            cp = 0
            for q in range(NQ):
                if loaded < NQ:
                    emit_load(loaded)
                    loaded += 1
                tk = tks.pop(q)
                o = obp.tile([128, OCOL], bf16, name="ob", tag="ob")
                for h in range(2):
                    for mi in range(3):
                        pss = [
                            psp.tile([128, 896], f32, name="ps", tag="ps")
                            for _ in range(2)
                        ]
                        for ki in range(2):
                            for bi in range(2):
                                c0 = ki * (QB * D) + (2 * h + bi) * D
                                for n0, nsz in _N_TILES:
                                    nc.tensor.matmul(
                                        pss[bi][:MP, n0 : n0 + nsz],
                                        w_sb[ki][:, mi * MP : (mi + 1) * MP],
                                        tk[:, c0 + n0 : c0 + n0 + nsz],
                                        start=(ki == 0),
                                        stop=(ki == 1),
                                    )
                        for bi in range(2):
                            b = 2 * h + bi
                            dst = o[:MP, (b * 3 + mi) * D : (b * 3 + mi + 1) * D]
                            if cp % 2 == 0:
                                nc.vector.tensor_copy(dst, pss[bi][:MP, :])
                            else:
                                nc.scalar.activation(
                                    dst,
                                    pss[bi][:MP, :],
                                    mybir.ActivationFunctionType.Copy,
                                )
                            cp += 1
                for j, (r0, rn) in enumerate(_Q_SPLITS):
                    nc.sync.dma_start(
                        out[4 * q + j, :rn, :], o[r0 : r0 + rn, :OCOL]
                    )

    nc.compile()
    _CACHE["nc"] = nc
    return nc


def _host_prep(tokens_full, large_mask, mid_mask):
    """Cast to bf16, pack tokens for 128-partition quad loads, build weights."""
    import ml_dtypes

    bf16 = ml_dtypes.bfloat16
    bsz = tokens_full.shape[0]
    tok_bf = np.asarray(tokens_full, np.float32).astype(bf16)

    # tokq[q, p, ki, b, d] = tok(4q+b, ki ? 97+p : p, d)
    t4 = tok_bf.reshape(bsz // QB, QB, T, D)
    tokq = np.empty((bsz // QB, KC, 2, QB, D), bf16)
    tokq[:, :, 0] = t4[:, :, 0:KC].transpose(0, 2, 1, 3)
    tokq[:, :, 1] = t4[:, :, KB0:T].transpose(0, 2, 1, 3)
    tokq = tokq.reshape(bsz // QB, KC, ROWE)

    W = np.zeros((G, T), np.float64)
    rows = np.arange(GL)
    for l in range(large_mask.shape[0]):
        np.add.at(W, (rows, large_mask[l]), 1.0 / LL)
    rows = GL + np.arange(GM)
    for l in range(mid_mask.shape[0]):
        np.add.at(W, (rows, mid_mask[l]), 1.0 / LM)
    W[G - 1, :] = 1.0 / T

    # Permute groups so m-tile mi, partition p <-> group 3p+mi.
    perm = np.concatenate([np.arange(mi, G, 3) for mi in range(3)])
    Wp = W[perm]  # [G, T]
    w01T = np.zeros((2, KC, G), np.float64)
    w01T[0] = Wp[:, 0:KC].T
    w01T[1] = Wp[:, KB0:T].T
    # Chunk B rows 0..30 are tokens 97..127, already counted in chunk A.
    w01T[1, : KC - KB0, :] = 0.0
    return tokq, w01T.astype(bf16)


def _in_maps(tokq, w01T, n_cores=N_CORES):
    qp = tokq.shape[0] // n_cores
    return [
        {
            "tokq": np.ascontiguousarray(tokq[c * qp : (c + 1) * qp]),
            "w01T": w01T,
        }
        for c in range(n_cores)
    ]


def _unpack_out(res_out):
    """[NQ*4, QROWS, OCOL] bf16 device layout -> [BP, G, D] f32."""
    arr = np.asarray(res_out).reshape(NQ * 4, QROWS, OCOL)
    full = np.empty((NQ, MP, QB, 3, D), np.float32)
    for q in range(NQ):
        for j, (r0, rn) in enumerate(_Q_SPLITS):
            full[q, r0 : r0 + rn] = (
                arr[4 * q + j, :rn].astype(np.float32).reshape(rn, QB, 3, D)
            )
    # [NQ, MP(perm'd groups /3), QB, 3, D] -> [BP, G(perm'd), D]
    permed = full.transpose(0, 2, 1, 3, 4).reshape(BP, G, D)
    return permed


def kernel(**inputs):
    from concourse import bass_utils

    tokens_full = np.ascontiguousarray(np.asarray(inputs["patch_tokens"], np.float32))
    large = np.asarray(inputs["large_mask"]).astype(np.int64)
    mid = np.asarray(inputs["mid_mask"]).astype(np.int64)
    tokq, w01T = _host_prep(tokens_full, large, mid)

    nc = _get_nc()
    res = bass_utils.run_bass_kernel_spmd(
        nc, _in_maps(tokq, w01T), core_ids=list(range(N_CORES))
    )
    return np.concatenate(
        [_unpack_out(res.results[c]["out"]) for c in range(N_CORES)], axis=0
    )
